# revision 15
# baseline (speedup 1.0000x reference)
"""Contrastive loss (InfoNCE, mean reduction) on 8 Trainium2 NeuronCores.

Reference computation (B=16384, D=64, fp32):
    a = embeddings_a / ||embeddings_a||_row ; b likewise
    sim = a @ b.T / 0.07
    loss = mean_i( logsumexp(sim[i, :]) - sim[i, i] )

Sharding: rows of `a` split across 8 cores (2048 rows each); every core gets
the full `b`. Each core computes its [2048, 16384] block of sim in PSUM.

v3 design (vs 341us baseline / 279us v2):
 - TensorE 64x128 row tiling: K=64 uses half the PE array, so a and b-hat are
   duplicated into SBUF partitions 64:127 and matmuls alternate between tiles
   T0 (partitions 0:63) and T8 (64:127), which stream concurrently. This
   halves the effective matmul issue time (PE runs cold at 1.2GHz here).
 - exp work split between ScalarE (ACT Exp + accum_out, per-partition scale
   AP = 1/(T*|a_i|)) and VectorE (Schraudolph int-bit exp via tensor_scalar,
   AP scalar) reading f32 PSUM. Row-sums of the V-cells go mostly to GPSIMD
   (tensor_reduce from SBUF), the rest to VectorE.
 - GPSIMD also does the b-side square (TT mul) and scale+duplicate cast (STT
   with broadcast APs); VectorE does norm-reduces and Quake rsqrt (bit trick
   + Newton) so ScalarE needs no Sqrt table: one ACT table set (exp+ln).
 - chunk 0 is prepped in 4-tile sub-chunks to start the main loop early;
   chunks prep one ahead of the main loop. XBAR transposes: chunk-0 subs + a
   on the scalar HWDGE queue, later chunks on the sync queue.
"""

import sys

sys.path.insert(0, "/opt/trn_rl_repo")

import numpy as np

B = 16384
D = 64
TEMP = 0.07
NCORES = 8
RPC = B // NCORES  # rows per core = 2048
NT_A = RPC // 128  # a tiles per core = 16
NCH = 8  # b chunks
TPC = 16  # b tiles per chunk

# per-chunk (S-width, n V-cells of 512): S-width + 512*nv == 2048
SPLITS = [(1024, 2), (1536, 1), (1536, 1), (1536, 1),
          (1024, 2), (1536, 1), (1536, 1), (1536, 1)]
NVTOT = sum(nv for _, nv in SPLITS)  # 10
GADD_MOD = 2  # every GADD_MOD-th 1-cell sum stays on VectorE; rest on GPSIMD

# Schraudolph exp: exp(x) ~= bitcast_f32(int32(x * 2^23/ln2 + (127*2^23 - C)))
SCHRAUDOLPH_C = 483000.0
S1 = float(2.0**23 / np.log(2.0))
S2 = float(127.0 * 2.0**23 - SCHRAUDOLPH_C)

_CACHE = {}


def _build_program():
    from contextlib import ExitStack

    import concourse.bacc as bacc
    import concourse.tile as tile
    from concourse import mybir

    f32 = mybir.dt.float32
    i32 = mybir.dt.int32
    bf16 = mybir.dt.bfloat16
    AF = mybir.ActivationFunctionType
    OP = mybir.AluOpType
    AX = mybir.AxisListType.X

    nc = bacc.Bacc("TRN2", target_bir_lowering=False, debug=False)
    a_ap = nc.dram_tensor("a", [RPC, D], f32, kind="ExternalInput").ap()
    b_ap = nc.dram_tensor("b", [B, D], f32, kind="ExternalInput").ap()
    bd_ap = nc.dram_tensor("bdiag", [RPC, D], f32, kind="ExternalInput").ap()
    out_ap = nc.dram_tensor("losses", [128, NT_A], f32, kind="ExternalOutput").ap()

    with ExitStack() as ctx:
        tc = ctx.enter_context(tile.TileContext(nc))
        big = ctx.enter_context(tc.tile_pool(name="big", bufs=1))
        prep = ctx.enter_context(tc.tile_pool(name="prep", bufs=4))
        expool = ctx.enter_context(tc.tile_pool(name="expool", bufs=8))
        spsum = ctx.enter_context(tc.tile_pool(name="spsum", bufs=2, space="PSUM"))
        vpsum = ctx.enter_context(tc.tile_pool(name="vpsum", bufs=1, space="PSUM"))

        # ---- persistent SBUF tensors ----
        b_nat = big.tile([128, 128, D], f32, tag="b_nat")
        a_nat = big.tile([128, NT_A, D], f32, tag="a_nat")
        bd_nat = big.tile([128, NT_A, D], f32, tag="bd_nat")
        # transposed, partition-duplicated: [d or d+64, tile, row]
        bT = big.tile([128, 128, 128], bf16, tag="bT")
        aT = big.tile([128, NT_A, 128], bf16, tag="aT")
        stage0 = big.tile([128, TPC, 128], bf16, tag="stage0")
        stage1 = big.tile([128, TPC, 128], bf16, tag="stage1")
        stage = [stage0, stage1]
        astage = big.tile([128, NT_A, 128], bf16, tag="astage")
        rb = big.tile([128, 128], f32, tag="rb")
        ra = big.tile([128, NT_A], f32, tag="ra")
        raS1 = big.tile([128, NT_A], f32, tag="raS1")
        rbd = big.tile([128, NT_A], f32, tag="rbd")
        diag = big.tile([128, NT_A], f32, tag="diag")
        rs_S = big.tile([128, NT_A, NCH], f32, tag="rs_S")
        rs_V = big.tile([128, NT_A, NVTOT], f32, tag="rs_V")
        nc.vector.memset(rs_V[:], 0)

        # ---- input DMAs (sync queue; chunk 0 split in 4 for early start) ----
        b_r = b_ap.rearrange("(t p) d -> p t d", p=128)
        nc.sync.dma_start(a_nat[:], a_ap.rearrange("(t p) d -> p t d", p=128))
        for s in range(4):
            nc.sync.dma_start(
                b_nat[:, s * 4 : (s + 1) * 4, :], b_r[:, s * 4 : (s + 1) * 4, :]
            )
        nc.sync.dma_start(bd_nat[:], bd_ap.rearrange("(t p) d -> p t d", p=128))
        for g in range(1, NCH):
            nc.sync.dma_start(
                b_nat[:, g * TPC : (g + 1) * TPC, :], b_r[:, g * TPC : (g + 1) * TPC, :]
            )
        acc_g = big.tile([128, NT_A, 512], f32, tag="acc_g")
        vps = vpsum.tile([128, 2, 512], f32, tag="vps")

        # ---- helpers ----
        QK = float(0x5F3759DF + 1)

        def rsqrt(dst, nsq, nt, pre_scale=None, iters=2):
            # dst = 1/sqrt(nsq * pre_scale): Quake seed + Newton (VectorE)
            if pre_scale is not None:
                nc.vector.tensor_scalar_mul(dst, nsq, pre_scale)
                x = dst
            else:
                x = nsq
            t = prep.tile([128, 16], i32, tag="qk_t")
            y = prep.tile([128, 16], f32, tag="qk_y")
            u = prep.tile([128, 16], f32, tag="qk_u")
            w = prep.tile([128, 16], f32, tag="qk_w")
            tn, yn, un, wn = t[:, :nt], y[:, :nt], u[:, :nt], w[:, :nt]
            nc.vector.tensor_scalar(
                tn, x.bitcast(i32), 1, 0, op0=OP.logical_shift_right, op1=OP.bitwise_not
            )
            nc.vector.tensor_scalar(
                yn.bitcast(i32), tn, int(QK), 0, op0=OP.add, op1=OP.add
            )
            for _ in range(iters):
                nc.vector.tensor_mul(un, yn, yn)
                nc.vector.scalar_tensor_tensor(wn, x, -0.5, un, op0=OP.mult, op1=OP.mult)
                nc.vector.scalar_tensor_tensor(yn, wn, 1.5, yn, op0=OP.add, op1=OP.mult)
            nc.vector.tensor_copy(dst, yn)

        def norms_sq(dst, src3d, nt, sq_eng):
            # dst[128, nt] = row sums of squares; square on sq_eng, reduce on V
            scr = prep.tile([128, 16, D], f32, tag="scr")
            sq_eng.tensor_mul(scr[:, :nt, :], src3d, src3d)
            nc.vector.tensor_reduce(dst, scr[:, :nt, :], axis=AX, op=OP.add)

        def prep_btiles(g, t_lo, t_hi, iters=1):
            # norms+rsqrt+scale/dup-cast+transpose for b tiles [t_lo, t_hi)
            nt = t_hi - t_lo
            gs = slice(t_lo, t_hi)
            nsq = prep.tile([128, 16], f32, tag="nsq")
            norms_sq(nsq[:, :nt], b_nat[:, gs, :], nt, nc.gpsimd)
            rsqrt(rb[:, gs], nsq[:, :nt], nt, iters=iters)
            st = stage[g % 2]
            so = slice(t_lo - g * TPC, t_hi - g * TPC)
            # write both partition-halves (columns 0:64 and 64:128)
            rb3 = rb[:, gs].unsqueeze(2).broadcast_to([128, nt, D])
            nc.vector.scalar_tensor_tensor(
                st[:, so, 0:D], b_nat[:, gs, :], 1.0, rb3, op0=OP.mult, op1=OP.mult
            )
            nc.vector.tensor_copy(st[:, so, D:], st[:, so, 0:D])
            xbar = nc.scalar if g == 0 else nc.sync
            xbar.dma_start_transpose(
                bT[:, gs, :], st[:, so, :].rearrange("p t d -> p (t d)")
            )

        # ---- a path first (cast + transpose), then chunk-0 sub-chunks ----
        nc.vector.tensor_copy(astage[:, :, 0:D], a_nat[:])
        nc.vector.tensor_copy(astage[:, :, D:], a_nat[:])
        nc.scalar.dma_start_transpose(aT[:], astage[:].rearrange("p t d -> p (t d)"))
        prep_btiles(0, 0, 4)
        # a-norms group 0 (it 0:8) -> ra needed by first ACT
        nsq_a = prep.tile([128, 16], f32, tag="nsq_a")
        norms_sq(nsq_a[:, 0:8], a_nat[:, 0:8, :], 8, nc.vector)
        rsqrt(ra[:, 0:8], nsq_a[:, 0:8], 8, pre_scale=TEMP * TEMP)
        nc.vector.tensor_scalar_mul(raS1[:, 0:8], ra[:, 0:8], S1)
        prep_btiles(0, 4, 8)
        prep_btiles(0, 8, 12)
        nc.gpsimd.memset(acc_g[:, 0:8, :], 0)
        nc.gpsimd.memset(acc_g[:, 8:16, :], 0)
        norms_sq(nsq_a[:, 8:16], a_nat[:, 8:16, :], 8, nc.vector)
        rsqrt(ra[:, 8:16], nsq_a[:, 8:16], 8, pre_scale=TEMP * TEMP)
        nc.vector.tensor_scalar_mul(raS1[:, 8:16], ra[:, 8:16], S1)
        prep_btiles(0, 12, 16)

        # ---- bd norms + diag (tail-only dependency) ----
        nsq_bd = prep.tile([128, 16], f32, tag="nsq_bd")
        norms_sq(nsq_bd[:], bd_nat[:], NT_A, nc.gpsimd)
        rsqrt(rbd[:], nsq_bd[:], NT_A)
        scr_d = prep.tile([128, NT_A, D], f32, tag="scr_d")
        nc.gpsimd.tensor_mul(scr_d[:], a_nat[:], bd_nat[:])
        nc.vector.tensor_reduce(diag[:], scr_d[:], axis=AX, op=OP.add)
        nc.vector.tensor_mul(diag[:], diag[:], ra[:])
        nc.vector.tensor_mul(diag[:], diag[:], rbd[:])

        # ---- main loop ----
        vcell_ct = 0
        for g in range(NCH):
            if g + 1 < NCH:
                prep_btiles(g + 1, (g + 1) * TPC, (g + 2) * TPC)
            ws, nv = SPLITS[g]
            t0 = g * TPC
            vbase = sum(SPLITS[gg][1] for gg in range(g))
            for it in range(NT_A):
                lhs = [aT[0:D, it, :], aT[64 : 64 + D, it, :]]
                tp = [(0, 0), (64, 0)]
                half = [slice(0, D), slice(64, 64 + D)]
                mm = 0
                ps = spsum.tile([128, 1536], f32, tag="ps")
                for k in range(ws // 512):
                    h = mm % 2
                    nc.tensor.matmul(
                        ps[:, k * 512 : (k + 1) * 512],
                        lhsT=lhs[h],
                        rhs=bT[half[h], t0 + k * 4 : t0 + (k + 1) * 4, :],
                        start=True,
                        stop=True,
                        tile_position=tp[h],
                    )
                    mm += 1
                nc.scalar.activation(
                    ps[:, :ws], ps[:, :ws], AF.Exp,
                    scale=ra[:, it : it + 1],
                    accum_out=rs_S[:, it, g : g + 1],
                )
                if nv == 2:
                    for v in range(2):
                        kt = t0 + (ws // 128) + v * 4
                        h = mm % 2
                        nc.tensor.matmul(
                            vps[:, v, :],
                            lhsT=lhs[h],
                            rhs=bT[half[h], kt : kt + 4, :],
                            start=True,
                            stop=True,
                            tile_position=tp[h],
                        )
                        mm += 1
                    # both slots drained by one TSP + one batched reduce
                    ex2 = expool.tile([128, 2, 512], i32, tag="ex2")
                    nc.vector.tensor_scalar(
                        ex2[:], vps[:], raS1[:, it : it + 1], S2, op0=OP.mult, op1=OP.add
                    )
                    nc.vector.tensor_reduce(
                        rs_V[:, it, vbase : vbase + 2],
                        ex2[:].bitcast(f32),
                        axis=AX,
                        op=OP.add,
                    )
                else:
                    kt = t0 + (ws // 128)
                    h = mm % 2
                    slot = vcell_ct % 2
                    nc.tensor.matmul(
                        vps[:, slot, :],
                        lhsT=lhs[h],
                        rhs=bT[half[h], kt : kt + 4, :],
                        start=True,
                        stop=True,
                        tile_position=tp[h],
                    )
                    mm += 1
                    ex = expool.tile([128, 512], i32, tag="ex")
                    nc.vector.tensor_scalar(
                        ex[:], vps[:, slot, :], raS1[:, it : it + 1], S2,
                        op0=OP.mult, op1=OP.add
                    )
                    if vcell_ct % GADD_MOD == 0:
                        nc.vector.tensor_reduce(
                            rs_V[:, it, vbase : vbase + 1],
                            ex[:].bitcast(f32),
                            axis=AX,
                            op=OP.add,
                        )
                    else:
                        nc.gpsimd.tensor_add(
                            acc_g[:, it, :], acc_g[:, it, :], ex[:].bitcast(f32)
                        )
                    vcell_ct += 1

        # ---- tail ----
        rowsum = big.tile([128, NT_A], f32, tag="rowsum")
        rowsum_v = big.tile([128, NT_A], f32, tag="rowsum_v")
        rowsum_g = big.tile([128, NT_A], f32, tag="rowsum_g")
        nc.vector.tensor_reduce(rowsum[:], rs_S[:], axis=AX, op=OP.add)
        nc.vector.tensor_reduce(rowsum_v[:], rs_V[:], axis=AX, op=OP.add)
        nc.vector.tensor_reduce(rowsum_g[:], acc_g[:], axis=AX, op=OP.add)
        nc.vector.tensor_add(rowsum[:], rowsum[:], rowsum_v[:])
        nc.vector.tensor_add(rowsum[:], rowsum[:], rowsum_g[:])
        lse = big.tile([128, NT_A], f32, tag="lse")
        nc.scalar.activation(lse[:], rowsum[:], AF.Ln)
        out_sb = big.tile([128, NT_A], f32, tag="out_sb")
        nc.vector.tensor_sub(out_sb[:], lse[:], diag[:])
        nc.sync.dma_start(out_ap[:], out_sb[:])

    nc.compile()
    return nc


def get_program():
    if "nc" not in _CACHE:
        _CACHE["nc"] = _build_program()
    return _CACHE["nc"]


def make_in_maps(a, b):
    return [
        {
            "a": np.ascontiguousarray(a[c * RPC : (c + 1) * RPC]),
            "b": b,
            "bdiag": np.ascontiguousarray(b[c * RPC : (c + 1) * RPC]),
        }
        for c in range(NCORES)
    ]


def kernel(embeddings_a, embeddings_b):
    from concourse.bass_utils import run_bass_kernel_spmd

    a = np.ascontiguousarray(np.asarray(embeddings_a, dtype=np.float32))
    b = np.ascontiguousarray(np.asarray(embeddings_b, dtype=np.float32))
    assert a.shape == (B, D) and b.shape == (B, D)

    nc = get_program()
    res = run_bass_kernel_spmd(nc, make_in_maps(a, b), core_ids=list(range(NCORES)))
    total = 0.0
    for c in range(NCORES):
        total += res.results[c]["losses"].astype(np.float64).sum()
    return np.float32(total / B)


# revision 16
# speedup vs baseline: 1.1529x; 1.1529x over previous
"""Contrastive loss (InfoNCE, mean reduction) on 8 Trainium2 NeuronCores.

Reference computation (B=16384, D=64, fp32):
    a = embeddings_a / ||embeddings_a||_row ; b likewise
    sim = a @ b.T / 0.07
    loss = mean_i( logsumexp(sim[i, :]) - sim[i, i] )

Sharding: rows of `a` split across 8 cores (2048 rows each); every core gets
the full `b`. Each core computes its [2048, 16384] block of sim in PSUM.

v3 design (vs 341us baseline / 279us v2):
 - TensorE 64x128 row tiling: K=64 uses half the PE array, so a and b-hat are
   duplicated into SBUF partitions 64:127 and matmuls alternate between tiles
   T0 (partitions 0:63) and T8 (64:127), which stream concurrently. This
   halves the effective matmul issue time (PE runs cold at 1.2GHz here).
 - exp work split between ScalarE (ACT Exp + accum_out, per-partition scale
   AP = 1/(T*|a_i|)) and VectorE (Schraudolph int-bit exp via tensor_scalar,
   AP scalar) reading f32 PSUM. Row-sums of the V-cells go mostly to GPSIMD
   (tensor_reduce from SBUF), the rest to VectorE.
 - GPSIMD also does the b-side square (TT mul) and scale+duplicate cast (STT
   with broadcast APs); VectorE does norm-reduces and Quake rsqrt (bit trick
   + Newton) so ScalarE needs no Sqrt table: one ACT table set (exp+ln).
 - chunk 0 is prepped in 4-tile sub-chunks to start the main loop early;
   chunks prep one ahead of the main loop. XBAR transposes: chunk-0 subs + a
   on the scalar HWDGE queue, later chunks on the sync queue.
"""

import sys

sys.path.insert(0, "/opt/trn_rl_repo")

import numpy as np

B = 16384
D = 64
TEMP = 0.07
NCORES = 8
RPC = B // NCORES  # rows per core = 2048
NT_A = RPC // 128  # a tiles per core = 16
NCH = 8  # b chunks
TPC = 16  # b tiles per chunk

# per-chunk (S-width, n V-cells of 512): S-width + 512*nv == 2048
SPLITS = [(1024, 2), (1536, 1), (1536, 1), (1536, 1),
          (1024, 2), (1536, 1), (1536, 1), (1536, 1)]
NVTOT = sum(nv for _, nv in SPLITS)  # 10
GADD_MOD = 2  # every GADD_MOD-th 1-cell sum stays on VectorE; rest on GPSIMD

# Schraudolph exp: exp(x) ~= bitcast_f32(int32(x * 2^23/ln2 + (127*2^23 - C)))
SCHRAUDOLPH_C = 483000.0
S1 = float(2.0**23 / np.log(2.0))
S2 = float(127.0 * 2.0**23 - SCHRAUDOLPH_C)

_CACHE = {}


def _build_program():
    from contextlib import ExitStack

    import concourse.bacc as bacc
    import concourse.tile as tile
    from concourse import mybir

    f32 = mybir.dt.float32
    i32 = mybir.dt.int32
    bf16 = mybir.dt.bfloat16
    AF = mybir.ActivationFunctionType
    OP = mybir.AluOpType
    AX = mybir.AxisListType.X

    nc = bacc.Bacc("TRN2", target_bir_lowering=False, debug=False)
    a_ap = nc.dram_tensor("a", [RPC, D], f32, kind="ExternalInput").ap()
    b_ap = nc.dram_tensor("b", [B, D], f32, kind="ExternalInput").ap()
    bd_ap = nc.dram_tensor("bdiag", [RPC, D], f32, kind="ExternalInput").ap()
    out_ap = nc.dram_tensor("losses", [128, NT_A], f32, kind="ExternalOutput").ap()

    with ExitStack() as ctx:
        tc = ctx.enter_context(tile.TileContext(nc))
        big = ctx.enter_context(tc.tile_pool(name="big", bufs=1))
        prep = ctx.enter_context(tc.tile_pool(name="prep", bufs=4))
        expool = ctx.enter_context(tc.tile_pool(name="expool", bufs=8))
        spsum = ctx.enter_context(tc.tile_pool(name="spsum", bufs=2, space="PSUM"))
        vpsum = ctx.enter_context(tc.tile_pool(name="vpsum", bufs=2, space="PSUM"))

        # ---- persistent SBUF tensors ----
        b_nat = big.tile([128, 128, D], f32, tag="b_nat")
        a_nat = big.tile([128, NT_A, D], f32, tag="a_nat")
        bd_nat = big.tile([128, NT_A, D], f32, tag="bd_nat")
        # transposed, partition-duplicated: [d or d+64, tile, row]
        bT = big.tile([128, 128, 128], bf16, tag="bT")
        aT = big.tile([128, NT_A, 128], bf16, tag="aT")
        stage0 = big.tile([128, TPC, 128], bf16, tag="stage0")
        stage1 = big.tile([128, TPC, 128], bf16, tag="stage1")
        stage = [stage0, stage1]
        astage = big.tile([128, NT_A, 128], bf16, tag="astage")
        rb = big.tile([128, 128], f32, tag="rb")
        ra = big.tile([128, NT_A], f32, tag="ra")
        raS1 = big.tile([128, NT_A], f32, tag="raS1")
        rbd = big.tile([128, NT_A], f32, tag="rbd")
        diag = big.tile([128, NT_A], f32, tag="diag")
        rs_S = big.tile([128, NT_A, NCH], f32, tag="rs_S")
        rs_V = big.tile([128, NT_A, NVTOT], f32, tag="rs_V")
        nc.vector.memset(rs_V[:], 0)

        # ---- input DMAs (sync queue; chunk 0 split in 4 for early start) ----
        b_r = b_ap.rearrange("(t p) d -> p t d", p=128)
        nc.sync.dma_start(a_nat[:], a_ap.rearrange("(t p) d -> p t d", p=128))
        for s in range(4):
            nc.sync.dma_start(
                b_nat[:, s * 4 : (s + 1) * 4, :], b_r[:, s * 4 : (s + 1) * 4, :]
            )
        nc.sync.dma_start(bd_nat[:], bd_ap.rearrange("(t p) d -> p t d", p=128))
        for g in range(1, NCH):
            nc.sync.dma_start(
                b_nat[:, g * TPC : (g + 1) * TPC, :], b_r[:, g * TPC : (g + 1) * TPC, :]
            )
        acc_g = big.tile([128, NT_A, 512], f32, tag="acc_g")
        nc.gpsimd.memset(acc_g[:], 0)

        # ---- helpers ----
        QK = float(0x5F3759DF + 1)

        def rsqrt(dst, nsq, nt, pre_scale=None, iters=2):
            # dst = 1/sqrt(nsq * pre_scale): Quake seed + Newton (VectorE)
            if pre_scale is not None:
                nc.vector.tensor_scalar_mul(dst, nsq, pre_scale)
                x = dst
            else:
                x = nsq
            t = prep.tile([128, 16], i32, tag="qk_t")
            y = prep.tile([128, 16], f32, tag="qk_y")
            u = prep.tile([128, 16], f32, tag="qk_u")
            w = prep.tile([128, 16], f32, tag="qk_w")
            tn, yn, un, wn = t[:, :nt], y[:, :nt], u[:, :nt], w[:, :nt]
            nc.vector.tensor_scalar(
                tn, x.bitcast(i32), 1, 0, op0=OP.logical_shift_right, op1=OP.bitwise_not
            )
            nc.vector.tensor_scalar(
                yn.bitcast(i32), tn, int(QK), 0, op0=OP.add, op1=OP.add
            )
            for _ in range(iters):
                nc.vector.tensor_mul(un, yn, yn)
                nc.vector.scalar_tensor_tensor(wn, x, -0.5, un, op0=OP.mult, op1=OP.mult)
                nc.vector.scalar_tensor_tensor(yn, wn, 1.5, yn, op0=OP.add, op1=OP.mult)
            nc.vector.tensor_copy(dst, yn)

        def norms_sq(dst, src3d, nt, sq_eng):
            # dst[128, nt] = row sums of squares; square on sq_eng, reduce on V
            scr = prep.tile([128, 16, D], f32, tag="scr")
            sq_eng.tensor_mul(scr[:, :nt, :], src3d, src3d)
            nc.vector.tensor_reduce(dst, scr[:, :nt, :], axis=AX, op=OP.add)

        def prep_btiles(g, t_lo, t_hi, iters=1):
            # norms+rsqrt+scale/dup-cast+transpose for b tiles [t_lo, t_hi)
            nt = t_hi - t_lo
            gs = slice(t_lo, t_hi)
            nsq = prep.tile([128, 16], f32, tag="nsq")
            norms_sq(nsq[:, :nt], b_nat[:, gs, :], nt, nc.gpsimd)
            rsqrt(rb[:, gs], nsq[:, :nt], nt, iters=iters)
            st = stage[g % 2]
            so = slice(t_lo - g * TPC, t_hi - g * TPC)
            # write both partition-halves (columns 0:64 and 64:128)
            rb3 = rb[:, gs].unsqueeze(2).broadcast_to([128, nt, D])
            nc.vector.scalar_tensor_tensor(
                st[:, so, 0:D], b_nat[:, gs, :], 1.0, rb3, op0=OP.mult, op1=OP.mult
            )
            nc.vector.tensor_copy(st[:, so, D:], st[:, so, 0:D])
            xbar = nc.scalar if g == 0 else nc.sync
            xbar.dma_start_transpose(
                bT[:, gs, :], st[:, so, :].rearrange("p t d -> p (t d)")
            )

        # ---- a path first (cast + transpose), then chunk-0 sub-chunks ----
        nc.vector.tensor_copy(astage[:, :, 0:D], a_nat[:])
        nc.vector.tensor_copy(astage[:, :, D:], a_nat[:])
        nc.scalar.dma_start_transpose(aT[:], astage[:].rearrange("p t d -> p (t d)"))
        prep_btiles(0, 0, 4)
        # a-norms group 0 (it 0:8) -> ra needed by first ACT
        nsq_a = prep.tile([128, 16], f32, tag="nsq_a")
        norms_sq(nsq_a[:, 0:8], a_nat[:, 0:8, :], 8, nc.vector)
        rsqrt(ra[:, 0:8], nsq_a[:, 0:8], 8, pre_scale=TEMP * TEMP)
        nc.vector.tensor_scalar_mul(raS1[:, 0:8], ra[:, 0:8], S1)
        prep_btiles(0, 4, 8)
        prep_btiles(0, 8, 12)
        norms_sq(nsq_a[:, 8:16], a_nat[:, 8:16, :], 8, nc.vector)
        rsqrt(ra[:, 8:16], nsq_a[:, 8:16], 8, pre_scale=TEMP * TEMP)
        nc.vector.tensor_scalar_mul(raS1[:, 8:16], ra[:, 8:16], S1)
        prep_btiles(0, 12, 16)

        # ---- bd norms + diag (tail-only dependency) ----
        nsq_bd = prep.tile([128, 16], f32, tag="nsq_bd")
        norms_sq(nsq_bd[:], bd_nat[:], NT_A, nc.gpsimd)
        rsqrt(rbd[:], nsq_bd[:], NT_A)
        scr_d = prep.tile([128, NT_A, D], f32, tag="scr_d")
        nc.gpsimd.tensor_mul(scr_d[:], a_nat[:], bd_nat[:])
        nc.vector.tensor_reduce(diag[:], scr_d[:], axis=AX, op=OP.add)
        nc.vector.tensor_mul(diag[:], diag[:], ra[:])
        nc.vector.tensor_mul(diag[:], diag[:], rbd[:])

        # ---- main loop ----
        vcell_ct = 0
        for g in range(NCH):
            if g + 1 < NCH:
                prep_btiles(g + 1, (g + 1) * TPC, (g + 2) * TPC)
            ws, nv = SPLITS[g]
            t0 = g * TPC
            vbase = sum(SPLITS[gg][1] for gg in range(g))
            for it in range(NT_A):
                lhs = [aT[0:D, it, :], aT[64 : 64 + D, it, :]]
                tp = [(0, 0), (64, 0)]
                half = [slice(0, D), slice(64, 64 + D)]
                mm = 0
                ps = spsum.tile([128, 1536], f32, tag="ps")
                for k in range(ws // 512):
                    h = mm % 2
                    nc.tensor.matmul(
                        ps[:, k * 512 : (k + 1) * 512],
                        lhsT=lhs[h],
                        rhs=bT[half[h], t0 + k * 4 : t0 + (k + 1) * 4, :],
                        start=True,
                        stop=True,
                        tile_position=tp[h],
                    )
                    mm += 1
                nc.scalar.activation(
                    ps[:, :ws], ps[:, :ws], AF.Exp,
                    scale=ra[:, it : it + 1],
                    accum_out=rs_S[:, it, g : g + 1],
                )
                for v in range(nv):
                    kt = t0 + (ws // 128) + v * 4
                    h = mm % 2
                    pv = vpsum.tile([128, 512], f32, tag="pv")
                    nc.tensor.matmul(
                        pv[:],
                        lhsT=lhs[h],
                        rhs=bT[half[h], kt : kt + 4, :],
                        start=True,
                        stop=True,
                        tile_position=tp[h],
                    )
                    mm += 1
                    ex = expool.tile([128, 512], i32, tag="ex")
                    nc.vector.tensor_scalar(
                        ex[:], pv[:], raS1[:, it : it + 1], S2, op0=OP.mult, op1=OP.add
                    )
                    if vcell_ct % GADD_MOD == 0:
                        nc.vector.tensor_reduce(
                            rs_V[:, it, vbase + v : vbase + v + 1],
                            ex[:].bitcast(f32),
                            axis=AX,
                            op=OP.add,
                        )
                    else:
                        nc.gpsimd.tensor_add(
                            acc_g[:, it, :], acc_g[:, it, :], ex[:].bitcast(f32)
                        )
                    vcell_ct += 1

        # ---- tail ----
        rowsum = big.tile([128, NT_A], f32, tag="rowsum")
        rowsum_v = big.tile([128, NT_A], f32, tag="rowsum_v")
        rowsum_g = big.tile([128, NT_A], f32, tag="rowsum_g")
        nc.vector.tensor_reduce(rowsum[:], rs_S[:], axis=AX, op=OP.add)
        nc.vector.tensor_reduce(rowsum_v[:], rs_V[:], axis=AX, op=OP.add)
        nc.vector.tensor_reduce(rowsum_g[:], acc_g[:], axis=AX, op=OP.add)
        nc.vector.tensor_add(rowsum[:], rowsum[:], rowsum_v[:])
        nc.vector.tensor_add(rowsum[:], rowsum[:], rowsum_g[:])
        lse = big.tile([128, NT_A], f32, tag="lse")
        nc.scalar.activation(lse[:], rowsum[:], AF.Ln)
        out_sb = big.tile([128, NT_A], f32, tag="out_sb")
        nc.vector.tensor_sub(out_sb[:], lse[:], diag[:])
        nc.sync.dma_start(out_ap[:], out_sb[:])

    nc.compile()
    return nc


def get_program():
    if "nc" not in _CACHE:
        _CACHE["nc"] = _build_program()
    return _CACHE["nc"]


def make_in_maps(a, b):
    return [
        {
            "a": np.ascontiguousarray(a[c * RPC : (c + 1) * RPC]),
            "b": b,
            "bdiag": np.ascontiguousarray(b[c * RPC : (c + 1) * RPC]),
        }
        for c in range(NCORES)
    ]


def kernel(embeddings_a, embeddings_b):
    from concourse.bass_utils import run_bass_kernel_spmd

    a = np.ascontiguousarray(np.asarray(embeddings_a, dtype=np.float32))
    b = np.ascontiguousarray(np.asarray(embeddings_b, dtype=np.float32))
    assert a.shape == (B, D) and b.shape == (B, D)

    nc = get_program()
    res = run_bass_kernel_spmd(nc, make_in_maps(a, b), core_ids=list(range(NCORES)))
    total = 0.0
    for c in range(NCORES):
        total += res.results[c]["losses"].astype(np.float64).sum()
    return np.float32(total / B)
